# revision 3
# baseline (speedup 1.0000x reference)
"""Trainium2 Bass kernel for nn_BaseAtt (attention pooling).

reference:
    target = target_feats @ W.T                      # [B, 128]
    alpha  = softmax(mask(nf @ target), axis=k)      # [B, 200]
    onf    = sum_k alpha * nf                        # [B, 128]
    onl    = sum_k alpha * nl                        # [B, 128]

Sharding: data-parallel over B across 8 cores (512 batches/core).

Per-core pipeline (block = 32 batches, DMA group = 16 batches):
  - target.T [128d, 512b] via 8 accumulated fp32r matmuls (host-transposed
    W.T and target_feats.T inputs).
  - per batch: PE-transpose nf[b] (k-part layout) -> nfT [128d, 200k] in SBUF.
  - alpha rows: per-batch matmul with zero-masked stationary columns
    (z[:, i, :] = targetT col i on diag col i, else 0) accumulating into one
    PSUM tile [32, 256] -> all 32 alpha rows land on partitions 0..31.
  - standard softmax in b-partition layout (logmask add, max-sub, ACT exp
    with accum_out sum, reciprocal, scale).
  - weighted sums: same masked-stationary trick with alpha.T columns against
    the k-partition nf|nl tiles, accumulating [32, 256] output rows.
All matmuls in fp32r (1 cyc/row at free>=256; ~16-bit effective mantissa).
"""

import numpy as np

B, K, D, FD = 4096, 200, 128, 1024
NCORES = 8
BC = B // NCORES          # 512 batches per core
BLK = 32                  # softmax / MM-accumulation block
GRP = 16                  # DMA group (2 per block)
K0, K1 = 128, K - 128     # k-chunk sizes (128 + 72)
NEG = -1.0e9              # mask fill for logits (exp -> 0)


def gen_kernel():
    import concourse.bacc as bacc
    import concourse.tile as tile
    from concourse import mybir

    f32 = mybir.dt.float32
    f32r = mybir.dt.float32r
    AX = mybir.AxisListType
    AF = mybir.ActivationFunctionType

    nc = bacc.Bacc()

    tft = nc.declare_dram_parameter("tft", [FD, BC], f32r, isOutput=False)
    wt = nc.declare_dram_parameter("wt", [FD, D], f32r, isOutput=False)
    nf = nc.declare_dram_parameter("nf", [BC, K, D], f32r, isOutput=False)
    nl = nc.declare_dram_parameter("nl", [BC, K, D], f32r, isOutput=False)
    lmask = nc.declare_dram_parameter("lmask", [BC, K], f32, isOutput=False)
    ident = nc.declare_dram_parameter("ident", [128, 128], f32r, isOutput=False)
    m32 = nc.declare_dram_parameter("m32", [128, BLK, BLK], f32r, isOutput=False)

    onf = nc.declare_dram_parameter("onf", [BC, D], f32, isOutput=True)
    onl = nc.declare_dram_parameter("onl", [BC, D], f32, isOutput=True)

    with tile.TileContext(nc) as tc:
        with (
            tc.tile_pool(name="const", bufs=1) as const,
            tc.tile_pool(name="xin", bufs=4) as xin,
            tc.tile_pool(name="nft", bufs=4) as nft,
            tc.tile_pool(name="sm", bufs=2) as sm,
            tc.tile_pool(name="zp", bufs=2) as zp,
            tc.tile_pool(name="outp", bufs=3) as outp,
            tc.tile_pool(name="pst", bufs=1, space="PSUM") as pst,
            tc.tile_pool(name="psnf", bufs=2, space="PSUM") as psnf,
            tc.tile_pool(name="psa", bufs=2, space="PSUM") as psa,
            tc.tile_pool(name="pso", bufs=2, space="PSUM") as pso,
            tc.tile_pool(name="psx", bufs=1, space="PSUM") as psx,
        ):
            # ---- setup: constants ----
            id_t = const.tile([128, 128], f32r)
            nc.sync.dma_start(out=id_t, in_=ident[:, :])
            m32_t = const.tile([128, BLK, BLK], f32r)
            nc.sync.dma_start(out=m32_t, in_=m32[:, :, :])
            wt_t = const.tile([128, 8, D], f32r)
            nc.sync.dma_start(
                out=wt_t, in_=wt.rearrange("(fb fp) d -> fp fb d", fp=128)
            )
            tft_t = const.tile([128, 8, BC], f32r)
            nc.sync.dma_start(
                out=tft_t, in_=tft.rearrange("(fb fp) b -> fp fb b", fp=128)
            )

            # ---- target.T = W @ tf.T : [128 d, BC b] ----
            ps_t = pst.tile([128, BC], f32)
            for fb in range(8):
                nc.tensor.matmul(
                    ps_t, wt_t[:, fb, :], tft_t[:, fb, :],
                    start=(fb == 0), stop=(fb == 7),
                )
            targetT = const.tile([128, BC], f32r)
            nc.vector.tensor_copy(out=targetT, in_=ps_t.bitcast(f32r))

            # ---- main loop over 16 blocks of 32 batches ----
            for bb in range(BC // BLK):
                b0 = bb * BLK

                # stationary mask-diag for alpha: z[p, i, c] = tT[p, b0+i]*(i==c)
                z_t = zp.tile([128, BLK, BLK], f32r, tag="z")
                nc.vector.tensor_mul(
                    out=z_t,
                    in0=targetT[:, b0 : b0 + BLK].unsqueeze(2).broadcast_to(
                        [128, BLK, BLK]
                    ),
                    in1=m32_t,
                )

                x0s, x1s = [], []
                ps_a = psa.tile([BLK, 256], f32)
                for gi in range(BLK // GRP):
                    g0 = b0 + gi * GRP
                    x0 = xin.tile([128, GRP, 256], f32r, tag="x0")
                    x1 = xin.tile([K1, GRP, 256], f32r, tag="x1")
                    nc.sync.dma_start(
                        out=x0[:, :, 0:128],
                        in_=nf[g0 : g0 + GRP, 0:K0, :].rearrange("g k d -> k g d"),
                    )
                    nc.sync.dma_start(
                        out=x0[:, :, 128:256],
                        in_=nl[g0 : g0 + GRP, 0:K0, :].rearrange("g k d -> k g d"),
                    )
                    nc.sync.dma_start(
                        out=x1[:, :, 0:128],
                        in_=nf[g0 : g0 + GRP, K0:K, :].rearrange("g k d -> k g d"),
                    )
                    nc.sync.dma_start(
                        out=x1[:, :, 128:256],
                        in_=nl[g0 : g0 + GRP, K0:K, :].rearrange("g k d -> k g d"),
                    )
                    x0s.append(x0)
                    x1s.append(x1)

                    for i in range(GRP):
                        bi = gi * GRP + i
                        # nfT via PE transpose: [128 d, 200 k]
                        ps_nfT = psnf.tile([128, 256], f32r, tag="pnft")
                        nc.tensor.transpose(
                            ps_nfT[:, 0:K0], x0[:, i, 0:128], id_t
                        )
                        nc.tensor.transpose(
                            ps_nfT[:, K0:K], x1[:, i, 0:128], id_t[:K1, :K1]
                        )
                        nfT_s = nft.tile([128, 256], f32r, tag="nfts")
                        if i % 2 == 0:
                            nc.vector.tensor_copy(
                                out=nfT_s[:, 0:K], in_=ps_nfT[:, 0:K]
                            )
                        else:
                            nc.scalar.copy(
                                out=nfT_s[:, 0:K], in_=ps_nfT[:, 0:K]
                            )
                        # alpha row bi accumulates into ps_a
                        nc.tensor.matmul(
                            ps_a, z_t[:, bi, :], nfT_s,
                            start=(bi == 0), stop=(bi == BLK - 1),
                        )

                # ---- softmax over k for 32 batches ----
                alpha_b = sm.tile([BLK, K], f32, tag="alpha")
                nc.vector.tensor_copy(out=alpha_b, in_=ps_a[:, 0:K])
                lm_t = sm.tile([BLK, K], f32, tag="lm")
                nc.sync.dma_start(out=lm_t, in_=lmask[b0 : b0 + BLK, :])
                aM = sm.tile([BLK, K], f32, tag="am")
                nc.vector.tensor_add(out=aM, in0=alpha_b, in1=lm_t)
                mx = sm.tile([BLK, 1], f32, tag="mx")
                nc.vector.reduce_max(out=mx, in_=aM, axis=AX.X)
                negmx = sm.tile([BLK, 1], f32, tag="negmx")
                nc.vector.tensor_scalar_mul(out=negmx, in0=mx, scalar1=-1.0)
                aE = sm.tile([BLK, K], f32, tag="ae")
                s_t = sm.tile([BLK, 1], f32, tag="s")
                nc.scalar.activation(
                    out=aE, in_=aM, func=AF.Exp, bias=negmx, scale=1.0,
                    accum_out=s_t,
                )
                rs = sm.tile([BLK, 1], f32, tag="rs")
                nc.vector.reciprocal(out=rs, in_=s_t)
                aN = sm.tile([BLK, K], f32r, tag="an")
                nc.vector.tensor_scalar_mul(out=aN, in0=aE, scalar1=rs)

                # alpha.T via PE transpose: [200 k, 32 b]
                ps_aT = psx.tile([128, 2 * BLK], f32r, tag="pat")
                nc.tensor.transpose(ps_aT[:, 0:BLK], aN[:, 0:K0], id_t[:BLK, :BLK])
                nc.tensor.transpose(
                    ps_aT[:K1, BLK : 2 * BLK], aN[:, K0:K], id_t[:BLK, :BLK]
                )
                aT0 = zp.tile([128, BLK], f32r, tag="at0")
                nc.vector.tensor_copy(out=aT0, in_=ps_aT[:, 0:BLK])
                aT1 = zp.tile([K1, BLK], f32r, tag="at1")
                nc.vector.tensor_copy(out=aT1, in_=ps_aT[:K1, BLK : 2 * BLK])

                # weighted-sum stationaries
                za0 = zp.tile([128, BLK, BLK], f32r, tag="za0")
                nc.vector.tensor_mul(
                    out=za0,
                    in0=aT0.unsqueeze(2).broadcast_to([128, BLK, BLK]),
                    in1=m32_t,
                )
                za1 = zp.tile([K1, BLK, BLK], f32r, tag="za1")
                nc.vector.tensor_mul(
                    out=za1,
                    in0=aT1.unsqueeze(2).broadcast_to([K1, BLK, BLK]),
                    in1=m32_t[:K1],
                )

                # ---- weighted sums: out rows [32, nf(128) | nl(128)] ----
                ps_o = pso.tile([BLK, 256], f32)
                for i in range(BLK):
                    x0 = x0s[i // GRP]
                    x1 = x1s[i // GRP]
                    nc.tensor.matmul(
                        ps_o, za0[:, i, :], x0[:, i % GRP, :],
                        start=(i == 0), stop=False,
                    )
                    nc.tensor.matmul(
                        ps_o, za1[:, i, :], x1[:, i % GRP, :],
                        start=False, stop=(i == BLK - 1),
                    )
                out_s = outp.tile([BLK, 256], f32, tag="outs")
                nc.vector.tensor_copy(out=out_s, in_=ps_o)
                nc.sync.dma_start(
                    out=onf[b0 : b0 + BLK, :], in_=out_s[:, 0:128]
                )
                nc.sync.dma_start(
                    out=onl[b0 : b0 + BLK, :], in_=out_s[:, 128:256]
                )

    nc.finalize()
    return nc


_NC_CACHE = None


def _get_nc():
    global _NC_CACHE
    if _NC_CACHE is None:
        _NC_CACHE = gen_kernel()
    return _NC_CACHE


def build_in_maps(target_feats, neighbor_feats, neighbor_label, hist_mask, W):
    target_feats = np.ascontiguousarray(target_feats, dtype=np.float32)
    neighbor_feats = np.ascontiguousarray(neighbor_feats, dtype=np.float32)
    neighbor_label = np.ascontiguousarray(neighbor_label, dtype=np.float32)
    W = np.ascontiguousarray(W, dtype=np.float32)

    wt_full = np.ascontiguousarray(W.T)                      # [FD, D]
    lmask_full = np.where(np.asarray(hist_mask) > 0, 0.0, NEG).astype(np.float32)
    ident = np.eye(128, dtype=np.float32)
    m32 = np.zeros((128, BLK, BLK), dtype=np.float32)
    for i in range(BLK):
        m32[:, i, i] = 1.0

    in_maps = []
    for c in range(NCORES):
        s = slice(c * BC, (c + 1) * BC)
        in_maps.append({
            "tft": np.ascontiguousarray(target_feats[s].T),  # [FD, BC]
            "wt": wt_full,
            "nf": neighbor_feats[s],
            "nl": neighbor_label[s],
            "lmask": lmask_full[s],
            "ident": ident,
            "m32": m32,
        })
    return in_maps


def kernel(target_feats, neighbor_feats, neighbor_label, hist_mask, W):
    from concourse.bass_utils import run_bass_kernel_spmd

    in_maps = build_in_maps(
        target_feats, neighbor_feats, neighbor_label, hist_mask, W
    )
    nc = _get_nc()
    res = run_bass_kernel_spmd(nc, in_maps, list(range(NCORES))).results

    onf = np.concatenate([res[c]["onf"] for c in range(NCORES)], axis=0)
    onl = np.concatenate([res[c]["onl"] for c in range(NCORES)], axis=0)
    return onf, onl
